# revision 15
# baseline (speedup 1.0000x reference)
"""MixedQLinear Trainium2 kernel (v2: 2D token x feature sharding).

Computation (per reference):
  x2 = x[0]                                  (M=4096, IN_F=4096) fp16
  int_x = x2[:, int_indices]                 (M, 3840)
  fp_x  = x2[:, fp_indices]                  (M, 256)
  per-token asym quant of int_x to int4:  scale=(mx-mn)/15, zero=mn
  q = round((int_x-zero)/scale) - 8          in [-8,7]
  out = scale*w_scale*(q @ w_int.T) + (zero+8*scale)*reduced_w + fp_x@fp_w.T + bias

Strategy: 2D sharding, 4 token groups x 2 feature groups on 8 cores.
Each core owns 1024 tokens x 2048 out features.  No collectives; the
per-token quantization is computed only by the 2 cores owning those
tokens (vs 8x replication in v1).

Device-side per core:
  - per 128-token tile: min/max stats, quantize via ScalarE activation
    (per-partition scale/bias) + fp32 magic-constant RNE rounding on DVE,
    alpha = mn + 8*scale appended as an extra column so it transposes
    into a [1,128] row for the rank-2 (alpha,ones)x(redw,bias) matmul,
  - DMA xbar transpose (128 x 3968) -> k-major q tiles, cast to fp8,
  - int path: q8 pair is the matmul stationary, raw-int fp8 weights are
    the moving operand (N=512), DoubleRow K=256/step, exact int arith,
  - fp path: fp8 DR matmul (256 fp features) + fp16 K=2 matmul for
    alpha*reduced_w + bias into a second psum,
  - combine: ot = (p0 * wsb) * scale_t + p1  (2 DVE ops).

Host side does only layout work: column gather, int4 unpack, fp8 casts,
sharding, and concat of per-core outputs.
"""

import os
import sys

import numpy as np

for _p in ("/opt/trn_rl_repo",):
    if _p not in sys.path and os.path.isdir(_p):
        sys.path.insert(0, _p)

TOKENS = 4096
IN_F = 4096
OUT_F = 4096
FP_F = 256
INT_F = IN_F - FP_F          # 3840
NCORES = 8
TG = 4                       # token groups
FG = 2                       # feature groups
TPC = TOKENS // TG           # 1024 tokens per core
OPC = OUT_F // FG            # 2048 out features per core
NT = TPC // 128              # 8 token tiles per core
KE = INT_F // 128            # 30 k-tiles
KP = (KE + 1) * 128          # 3968 (alpha col + zero pad)
NH = 2                       # feature halves (1024 each = 2 psum banks)
HW = OPC // NH               # 1024
C_MAGIC = 12582912.0         # 1.5*2^23: fp32 add/sub forces RNE-to-integer

_PROGRAM = None
LAST_RESULTS = None


def _ensure_ntff_hook():
    """Install the axon NTFF profiling hook if the image's antenv lacks it.

    Best-effort: profiling only; compile/run work without it.
    """
    import contextlib
    import ctypes
    import types

    try:
        try:
            import antenv.axon_hooks as hooks_mod
        except ImportError:
            import antenv

            hooks_mod = types.ModuleType("antenv.axon_hooks")
            _holder = {}
            hooks_mod.set_axon_ntff_profile_hook = (
                lambda hook: _holder.__setitem__("hook", hook))
            hooks_mod.get_axon_ntff_profile_hook = (
                lambda: _holder.get("hook"))
            sys.modules["antenv.axon_hooks"] = hooks_mod
            antenv.axon_hooks = hooks_mod

        if hooks_mod.get_axon_ntff_profile_hook() is not None:
            return
        so_path = "/opt/axon/libaxon_pjrt.so"
        if not os.path.exists(so_path):
            return
        lib = ctypes.CDLL(so_path)
        if not hasattr(lib, "axon_start_nrt_profile"):
            return
        lib.axon_start_nrt_profile.argtypes = [
            ctypes.POINTER(ctypes.c_int64), ctypes.c_size_t]
        lib.axon_start_nrt_profile.restype = ctypes.c_int64
        lib.axon_stop_nrt_profile.argtypes = [ctypes.c_char_p]
        lib.axon_stop_nrt_profile.restype = ctypes.c_int64

        @contextlib.contextmanager
        def _hook(output_dir, device_ids):
            import jax

            jax.devices()
            if device_ids:
                ids = (ctypes.c_int64 * len(device_ids))(*device_ids)
                rc = lib.axon_start_nrt_profile(ids, len(device_ids))
            else:
                rc = lib.axon_start_nrt_profile(None, 0)
            if rc != 0:
                raise RuntimeError(f"axon_start_nrt_profile rc={rc}")
            try:
                yield
            finally:
                n = lib.axon_stop_nrt_profile(str(output_dir).encode())
                print(f"ntff profile: {n} file(s) written to {output_dir}")

        hooks_mod.set_axon_ntff_profile_hook(_hook)
    except Exception:
        pass


def _build_program():
    import concourse.mybir as mybir
    import concourse.tile as tile
    from concourse import bacc

    f16 = mybir.dt.float16
    f32 = mybir.dt.float32
    f8 = mybir.dt.float8e4
    Alu = mybir.AluOpType
    DR = mybir.MatmulPerfMode.DoubleRow

    nc = bacc.Bacc(None, target_bir_lowering=False)

    x_int = nc.dram_tensor("x_int", [TPC, INT_F], f16, kind="ExternalInput")
    fpx8_d = nc.dram_tensor("fpx8", [128, 2, TPC], f8, kind="ExternalInput")
    wq_d = nc.dram_tensor("wq", [128, KE, OPC], f8, kind="ExternalInput")
    fpw8_d = nc.dram_tensor("fpw8", [128, 2, OPC], f8, kind="ExternalInput")
    rb_d = nc.dram_tensor("rb", [2, OPC], f16, kind="ExternalInput")
    wsb_d = nc.dram_tensor("wsb", [128, OPC], f32, kind="ExternalInput")
    out_d = nc.dram_tensor("out", [TPC, OPC], f16, kind="ExternalOutput")

    with tile.TileContext(nc) as tc:
        with tc.tile_pool(name="consts", bufs=1) as consts, \
             tc.tile_pool(name="xin", bufs=4) as xin, \
             tc.tile_pool(name="y0p", bufs=2) as y0p, \
             tc.tile_pool(name="qap", bufs=2) as qap, \
             tc.tile_pool(name="qtp", bufs=2) as qtp, \
             tc.tile_pool(name="qt8", bufs=5) as qt8, \
             tc.tile_pool(name="jnk", bufs=1) as jnk, \
             tc.tile_pool(name="stp", bufs=8) as stp, \
             tc.tile_pool(name="mp", bufs=2) as mp, \
             tc.tile_pool(name="otp", bufs=3) as otp, \
             tc.tile_pool(name="ps0", bufs=2, space="PSUM") as ps0, \
             tc.tile_pool(name="ps1", bufs=2, space="PSUM") as ps1:

            # === Constants / weights.  ppack[:, 4r:4r+4] per token tile r
            # holds [scale, rs, bq, alpha] as fp32 columns.
            ppack = consts.tile([128, 4 * NT], f32)
            arow = consts.tile([2, TPC], f16)
            # row 1 = ones (bias matmul); row 0 overwritten with alpha_t
            # per tile (engine APs must start at partition 0/32/64/96)
            nc.vector.memset(arow[0:2, :], 1.0)

            wq_s = consts.tile([128, KE, OPC], f8)
            fpx8_s = consts.tile([128, 2, TPC], f8)
            fpw8_s = consts.tile([128, 2, OPC], f8)
            rb_s = consts.tile([2, OPC], f16)
            wsb_s = consts.tile([128, OPC], f32)

            def load_fpx():
                nc.gpsimd.dma_start(out=fpx8_s[:, :, :], in_=fpx8_d[:, :, :])

            def load_small_consts():
                nc.gpsimd.dma_start(out=fpw8_s[:, :, :], in_=fpw8_d[:, :, :])
                nc.gpsimd.dma_start(out=rb_s[:, :], in_=rb_d[:, :])
                nc.gpsimd.dma_start(out=wsb_s[:, :], in_=wsb_d[:, :])

            def load_wq_pair(e):
                """Per-k-pair weight chunk so the first matmuls aren't gated
                on the full 7.5MB weight transfer."""
                nc.gpsimd.dma_start(
                    out=wq_s[:, 2 * e:2 * e + 2, :],
                    in_=wq_d[:, 2 * e:2 * e + 2, :])

            def param(r, v):
                """AP of param v for token tile r: [scale, rs, bq, alpha]."""
                return ppack[:, 4 * r + v:4 * r + v + 1]

            def producer(r):
                xt = xin.tile([128, INT_F], f16)
                nc.gpsimd.dma_start(
                    out=xt[:, :], in_=x_int[r * 128:(r + 1) * 128, :])
                # per-token stats: 2-level tree is ~2.4x faster than a direct
                # 3840-wide tensor_reduce on DVE
                mn = stp.tile([128, 1], f32, tag="mn")
                mx = stp.tile([128, 1], f32, tag="mx")
                a1 = jnk.tile([128, 1920], f16, tag="a1")
                a2 = jnk.tile([128, 960], f16, tag="a2")
                nc.vector.tensor_tensor(
                    out=a1[:, :], in0=xt[:, :1920], in1=xt[:, 1920:],
                    op=Alu.min)
                nc.vector.tensor_tensor(
                    out=a2[:, :], in0=a1[:, :960], in1=a1[:, 960:], op=Alu.min)
                nc.vector.tensor_reduce(
                    out=mn[:, :], in_=a2[:, :], axis=mybir.AxisListType.X,
                    op=Alu.min)
                b1 = jnk.tile([128, 1920], f16, tag="b1")
                b2 = jnk.tile([128, 960], f16, tag="b2")
                nc.vector.tensor_tensor(
                    out=b1[:, :], in0=xt[:, :1920], in1=xt[:, 1920:],
                    op=Alu.max)
                nc.vector.tensor_tensor(
                    out=b2[:, :], in0=b1[:, :960], in1=b1[:, 960:], op=Alu.max)
                nc.vector.tensor_reduce(
                    out=mx[:, :], in_=b2[:, :], axis=mybir.AxisListType.X,
                    op=Alu.max)
                d = stp.tile([128, 1], f32, tag="d")
                nc.vector.tensor_sub(d[:, :], mx[:, :], mn[:, :])
                nc.vector.tensor_scalar(
                    out=param(r, 0), in0=d[:, :],
                    scalar1=1.0 / 15.0, scalar2=1e-8, op0=Alu.mult, op1=Alu.max)
                nc.vector.reciprocal(param(r, 1), param(r, 0))
                tt = stp.tile([128, 1], f32, tag="tt")
                nc.vector.tensor_mul(tt[:, :], mn[:, :], param(r, 1))
                nc.vector.tensor_scalar(
                    out=param(r, 2), in0=tt[:, :],
                    scalar1=-1.0, scalar2=-8.0, op0=Alu.mult, op1=Alu.add)
                t8 = stp.tile([128, 1], f32, tag="t8")
                nc.vector.tensor_scalar(
                    out=t8[:, :], in0=param(r, 0),
                    scalar1=8.0, scalar2=None, op0=Alu.mult)
                nc.vector.tensor_add(param(r, 3), t8[:, :], mn[:, :])

                # quantize: y0 = x*rs + bq (fp32), qa = (y0+C)-C (RNE round)
                # processed in halves to halve y0's SBUF footprint
                qa = qap.tile([128, KP], f16)
                for hh in range(2):
                    cs = slice(hh * (INT_F // 2), (hh + 1) * (INT_F // 2))
                    y0 = y0p.tile([128, INT_F // 2], f32)
                    nc.scalar.activation(
                        out=y0[:, :], in_=xt[:, cs],
                        func=mybir.ActivationFunctionType.Identity,
                        bias=param(r, 2), scale=param(r, 1))
                    nc.vector.tensor_scalar(
                        out=qa[:, cs], in0=y0[:, :], scalar1=C_MAGIC,
                        scalar2=-C_MAGIC, op0=Alu.add, op1=Alu.add)
                nc.vector.tensor_copy(
                    out=qa[:, INT_F:INT_F + 1], in_=param(r, 3))
                nc.vector.memset(qa[:, INT_F + 1:], 0.0)
                # k-major transpose via DMA xbar: qtb[p,e,t] = qa[t,e*128+p]
                qtb = qtp.tile([128, KE + 1, 128], f16)
                nc.sync.dma_start_transpose(out=qtb[:, :, :], in_=qa[:, :])
                # fp8 copy of the q part (exact: ints in [-8,7]); DVE CAST is
                # ~4.5x faster than ScalarE COPY here
                q8 = qt8.tile([128, KE, 128], f8)
                nc.vector.tensor_copy(out=q8[:, :, :], in_=qtb[:, :KE, :])
                # alpha row rides the transpose: qtb[0, KE, t] = alpha_t
                nc.vector.tensor_copy(
                    out=arow[0:1, r * 128:(r + 1) * 128], in_=qtb[0:1, KE, :])
                return q8

            def consumer(r, h, q8):
                cols = slice(h * HW, (h + 1) * HW)
                t0 = r * 128
                # int path: q stationary, weights moving, 2 psum banks
                p0 = ps0.tile([128, HW], f32)
                for e in range(KE // 2):
                    for n in range(HW // 512):
                        nc.tensor.matmul(
                            p0[:, n * 512:(n + 1) * 512],
                            q8[:, 2 * e:2 * e + 2, :],
                            wq_s[:, 2 * e:2 * e + 2,
                                 h * HW + n * 512:h * HW + (n + 1) * 512],
                            start=(e == 0), stop=(e == KE // 2 - 1),
                            perf_mode=DR)
                # fp path + alpha*redw + bias into p1
                p1 = ps1.tile([128, HW], f32)
                for n in range(HW // 512):
                    nc.tensor.matmul(
                        p1[:, n * 512:(n + 1) * 512],
                        fpx8_s[:, :, t0:t0 + 128],
                        fpw8_s[:, :, h * HW + n * 512:h * HW + (n + 1) * 512],
                        start=True, stop=False, perf_mode=DR)
                for n in range(HW // 512):
                    nc.tensor.matmul(
                        p1[:, n * 512:(n + 1) * 512],
                        arow[:, t0:t0 + 128],
                        rb_s[:, h * HW + n * 512:h * HW + (n + 1) * 512],
                        start=False, stop=True)
                # combine: out = (p0*wsb)*scale_t + p1
                m = mp.tile([128, HW], f32)
                nc.vector.tensor_mul(m[:, :], p0[:, :], wsb_s[:, cols])
                ot = otp.tile([128, HW], f16)
                nc.vector.affine_then_add(
                    out=ot[:, :], in0=m[:, :], in1=p1[:, :],
                    scale=param(r, 0), bias=0.0)
                nc.gpsimd.dma_start(out=out_d[t0:t0 + 128, cols], in_=ot[:, :])

            # Load order on the gpsimd queue: x(0), first wq pairs, small
            # consts, then the rest of the wq pairs interleaved with the
            # remaining producers so the first matmuls are gated only on
            # ~2MB of transfers, not the full 10MB.
            LA = 4
            made = {}
            made[0] = producer(0)
            load_wq_pair(0)
            load_wq_pair(1)
            load_fpx()
            made[1] = producer(1)
            for e in range(2, 5):
                load_wq_pair(e)
            made[2] = producer(2)
            load_small_consts()
            for e in range(5, 10):
                load_wq_pair(e)
            made[3] = producer(3)
            for e in range(10, KE // 2):
                load_wq_pair(e)
            for r in range(NT):
                q8 = made.pop(r)
                for h in range(NH):
                    consumer(r, h, q8)
                # producer AFTER the consumers: its DVE ops (which may wait
                # on a lagging x DMA) must not head-of-line-block the
                # combines in the DVE FIFO, else PSUM never frees and the
                # PE stalls.
                if r + LA < NT:
                    made[r + LA] = producer(r + LA)

    nc.finalize()
    return nc


def _get_program():
    global _PROGRAM
    if _PROGRAM is None:
        _PROGRAM = _build_program()
    return _PROGRAM


def _unpack_i4(w_packed):
    """(out, INT_F//2) uint8 -> (out, INT_F) int8; col 2k=low nibble, 2k+1=high."""
    lo = (w_packed & 0x0F).astype(np.int8)
    hi = ((w_packed >> 4) & 0x0F).astype(np.int8)
    lo = np.where(lo >= 8, lo - 16, lo)
    hi = np.where(hi >= 8, hi - 16, hi)
    w = np.empty((w_packed.shape[0], w_packed.shape[1] * 2), dtype=np.int8)
    w[:, 0::2] = lo
    w[:, 1::2] = hi
    return w


def _prep_inputs(x, int_weight, weights_scales, reduced_w, fp_weight, bias,
                 int_indices, fp_indices):
    import ml_dtypes
    f8np = ml_dtypes.float8_e4m3

    x2 = np.asarray(x, dtype=np.float16)[0]
    int_idx = np.asarray(int_indices).astype(np.int64)
    fp_idx = np.asarray(fp_indices).astype(np.int64)

    x_int = np.ascontiguousarray(x2[:, int_idx])            # (M, 3840) f16
    fp_xT = np.ascontiguousarray(x2[:, fp_idx].T)           # (256, M) f16

    w_int = _unpack_i4(np.asarray(int_weight))              # (OUT_F, 3840) int8
    wsc = np.asarray(weights_scales).astype(np.float32)     # (OUT_F, 1)
    redw = np.asarray(reduced_w).astype(np.float16)         # (1, OUT_F)
    fpW = np.asarray(fp_weight).astype(np.float16)          # (OUT_F, 256)
    b = np.asarray(bias).astype(np.float16)                 # (OUT_F,)

    # per feature group: weights layouts
    fg_maps = []
    for g in range(FG):
        rows = slice(g * OPC, (g + 1) * OPC)
        wq = np.ascontiguousarray(
            w_int[rows].T.reshape(KE, 128, OPC).transpose(1, 0, 2)
        ).astype(f8np)                                      # [128, KE, OPC]
        fpw8 = np.ascontiguousarray(
            fpW[rows].T.reshape(2, 128, OPC).transpose(1, 0, 2)
        ).astype(f8np)                                      # [128, 2, OPC]
        rb = np.stack([redw[0, rows], b[rows]]).astype(np.float16)  # [2, OPC]
        wsb = np.broadcast_to(
            wsc[rows, 0][None, :], (128, OPC)).astype(np.float32).copy()
        fg_maps.append({"wq": wq, "fpw8": fpw8, "rb": rb, "wsb": wsb})

    in_maps = []
    for c in range(NCORES):
        tg, g = divmod(c, FG)
        toks = slice(tg * TPC, (tg + 1) * TPC)
        x_c = np.ascontiguousarray(x_int[toks])             # (TPC, 3840) f16
        fpx8 = np.ascontiguousarray(
            fp_xT[:, toks].reshape(2, 128, TPC).transpose(1, 0, 2)
        ).astype(f8np)                                      # [128, 2, TPC]
        m = {"x_int": x_c, "fpx8": fpx8}
        m.update(fg_maps[g])
        in_maps.append(m)
    return in_maps


def kernel(x, int_weight, weights_scales, reduced_w, fp_weight, bias,
           int_indices, fp_indices):
    global LAST_RESULTS
    from concourse.bass_utils import run_bass_kernel_spmd

    _ensure_ntff_hook()
    in_maps = _prep_inputs(x, int_weight, weights_scales, reduced_w,
                           fp_weight, bias, int_indices, fp_indices)
    nc = _get_program()
    res = run_bass_kernel_spmd(nc, in_maps, core_ids=list(range(NCORES)))
    LAST_RESULTS = res
    out = np.empty((TOKENS, OUT_F), dtype=np.float16)
    for c in range(NCORES):
        tg, g = divmod(c, FG)
        out[tg * TPC:(tg + 1) * TPC, g * OPC:(g + 1) * OPC] = \
            res.results[c]["out"]
    return out[None]
